# revision 69
# baseline (speedup 1.0000x reference)
"""Trainium2 Bass kernel for nn_AutoReg (4-layer dense transformer, teacher forcing).

Sharding across 8 NeuronCores: data-parallel over batch (B=4 -> 4 core pairs),
sequence-split within each pair with INTERLEAVED row blocks (even core owns
S-blocks {0,2,4,6}, odd owns {1,3,5,7}).  Ascending interleaved blocks give
the local and remote attention passes the identical causal suffix structure
[512,384,256,128] on every core, so one SPMD program skips ~38% of
score/exp/AV work (even cores waste one fully-masked 128-col sub-block per
remote key slot; the masks are per-core data).

Per layer the cores exchange only the 0.5MB fp8e4m3-cast LN1 output (gpsimd
casting DMAs + pair AllGather) — the collective starts before the K/V
projections and hides entirely under own-row K/V/Q + the local attention
pass; each core then re-projects the partner's K/V locally.

Math: bf16 matmuls with fp32 PSUM accumulation; LayerNorm, softmax and the
residual stream fp32 (fp8 touches only the xT wire).  Scores are computed
transposed (sT[rk, rq]) so the softmax denominator comes out of the AV
matmul via an appended ones column; the per-rstd Ln/Exp pair is batched per
LayerNorm to avoid ACT spline-table thrashing; softmax normalization uses a
fast approximate reciprocal + a gpsimd partition_broadcast.  Engine split:
PE matmuls, ACT exp/relu/copies, DVE LN/combine/normalize, GPSIMD masks
(remote pass), reciprocal broadcast, b2 fold and all casting DMAs — nothing
that must run during the collective sits behind it on the Pool queue.
"""

import numpy as np
import ml_dtypes

import concourse.bass as bass
import concourse.bacc as bacc
import concourse.mybir as mybir
import concourse.tile as tile
from concourse.bass import ds
from concourse.bass_utils import run_bass_kernel_spmd
from concourse.masks import make_identity

# Model dims (hardcoded per the problem spec)
L, B, S, D, H, F = 4, 4, 1024, 1024, 16, 4096
V1, V2, OUT = 32, 16, 50
HD = D // H            # 64
NCORES = 8
RLOC = 512             # local rows per core
NRB = RLOC // 128      # 4 local row blocks
NC_ = D // 128         # 8 D-chunks
NFO = F // 128         # 32 F-chunks
SCALE = 1.0 / np.sqrt(HD)

# global row-block assignment per parity (interleaved): parity 0 (even
# cores) own blocks [0,2,4,6]; parity 1 own [1,3,5,7].  With ascending
# interleaved blocks, BOTH the local and remote attention passes have the
# same causal suffix structure [512,384,256,128] on every core (even cores
# compute one fully-masked 128-col sub-block per remote key slot; odd cores
# use all of it), so the SPMD program can skip ~38% of score/exp/AV work.
BLOCKS = {0: [0, 2, 4, 6], 1: [1, 3, 5, 7]}

BF = mybir.dt.bfloat16
F32 = mybir.dt.float32
F8 = mybir.dt.float8e4

KSEG = D * RLOC              # elems: kT region of one core's kv block
VSEG = RLOC * D              # elems: v region
SEG = KSEG + VSEG            # elems per rank in the AllGather


def _build_program():
    nc = bacc.Bacc("TRN2", target_bir_lowering=False)

    # ---- DRAM parameters (per-core inputs) ----
    eat_in = nc.declare_dram_parameter("eat", [64, RLOC], BF, isOutput=False)
    wa_in = nc.declare_dram_parameter("wa", [64, D], BF, isOutput=False)
    pos_in = nc.declare_dram_parameter("pos", [RLOC, D], F32, isOutput=False)
    masks_in = nc.declare_dram_parameter("masks", [128, 8, 128], BF, isOutput=False)
    wq_in = nc.declare_dram_parameter("wq", [L * D, D], BF, isOutput=False)
    wk_in = nc.declare_dram_parameter("wk", [L * D, D], BF, isOutput=False)
    wv_in = nc.declare_dram_parameter("wv", [L * D, D], BF, isOutput=False)
    wo_in = nc.declare_dram_parameter("wo", [L * D, D], BF, isOutput=False)
    w1_in = nc.declare_dram_parameter("w1", [L * D, F], BF, isOutput=False)
    w2_in = nc.declare_dram_parameter("w2", [L * F, D], BF, isOutput=False)
    b1_in = nc.declare_dram_parameter("b1", [L * F], F32, isOutput=False)
    b2_in = nc.declare_dram_parameter("b2", [L * D], F32, isOutput=False)
    ln1g_in = nc.declare_dram_parameter("ln1g", [L * D], F32, isOutput=False)
    ln1b_in = nc.declare_dram_parameter("ln1b", [L * D], F32, isOutput=False)
    ln2g_in = nc.declare_dram_parameter("ln2g", [L * D], F32, isOutput=False)
    ln2b_in = nc.declare_dram_parameter("ln2b", [L * D], F32, isOutput=False)
    lnfg_in = nc.declare_dram_parameter("lnfg", [D], F32, isOutput=False)
    lnfb_in = nc.declare_dram_parameter("lnfb", [D], F32, isOutput=False)
    wd_in = nc.declare_dram_parameter("wd", [D, OUT], BF, isOutput=False)
    bd_in = nc.declare_dram_parameter("bd", [OUT], F32, isOutput=False)
    out_p = nc.declare_dram_parameter("out", [RLOC, OUT], F32, isOutput=True)

    def bcast_ap(src_ap, p=128):
        """Partition-broadcast view of a 1-D DRAM AP."""
        return bass.AP(tensor=src_ap.tensor, offset=src_ap.offset,
                       ap=[[0, p]] + [list(x) for x in src_ap.ap])

    AF = mybir.ActivationFunctionType
    ALU = mybir.AluOpType

    with tile.TileContext(nc) as tc:
        with tc.tile_pool(name="res", bufs=1) as res, \
             tc.tile_pool(name="wbig", bufs=2) as wbig, \
             tc.tile_pool(name="yt", bufs=1) as ytp, \
             tc.tile_pool(name="xt", bufs=2) as xtp, \
             tc.tile_pool(name="expp", bufs=2) as expp, \
             tc.tile_pool(name="xc", bufs=2) as xcp, \
             tc.tile_pool(name="prm", bufs=2) as prm, \
             tc.tile_pool(name="sm", bufs=4) as sm, \
             tc.tile_pool(name="dr", bufs=1, space="DRAM") as dr, \
             tc.tile_pool(name="ps_s", bufs=2, space="PSUM") as ps_s, \
             tc.tile_pool(name="ps_av", bufs=2, space="PSUM") as ps_av:

            # ---- resident tiles ----
            h_sb = res.tile([128, NRB, D], F32)            # residual stream
            kst = res.tile([128, NC_, RLOC], BF)           # own K^T
            kT_rem = res.tile([128, NC_, RLOC], BF)        # partner K^T
            v_loc = res.tile([128, NRB, H, HD + 1], BF)    # own V + ones col
            v_rem = res.tile([128, NRB, H, HD + 1], BF)    # partner V + ones col
            qT_sb = res.tile([128, NC_, RLOC], BF)
            oT_sb = res.tile([128, NC_, RLOC], BF)
            p1_sb = res.tile([HD + 1, H, RLOC], BF)        # pass-1 partial (o|sum)
            eat_sb = res.tile([64, RLOC], BF)
            wa_sb = res.tile([64, D], BF)
            ident = res.tile([128, 128], F32)
            ones1 = res.tile([1, 64], BF)
            wd_sb = res.tile([128, NC_, OUT], BF)
            bd_bc = res.tile([128, OUT], F32)
            eps_sb = res.tile([128, 1], F32)

            make_identity(nc, ident)
            nc.vector.memset(eps_sb, 1e-6)
            nc.vector.memset(ones1, 1.0)
            nc.vector.memset(v_loc[:, :, :, HD:HD + 1], 1.0)
            nc.vector.memset(v_rem[:, :, :, HD:HD + 1], 1.0)
            nc.sync.dma_start(eat_sb, eat_in[:, :])
            nc.sync.dma_start(wa_sb, wa_in[:, :])
            nc.sync.dma_start(wd_sb, wd_in.rearrange("(c p) n -> p c n", p=128))
            nc.sync.dma_start(bd_bc, bcast_ap(bd_in[:]))

            # dynamic base: partner's segment offset in the pair AllGather
            # output (register on the Pool engine — the fetches are gpsimd
            # casting DMAs)
            pid = nc.gpsimd.partition_id()
            par = pid - (pid // 2) * 2
            rem_base = (1 - par) * KSEG

            # ---- embedding: h = EaT^T @ Wa + pos ----
            pos_sb = wbig.tile([128, NRB, D], F32, tag="w2mb")
            nc.sync.dma_start(pos_sb, pos_in.rearrange("(rb p) d -> p rb d", p=128))
            for rb in range(NRB):
                for o2 in range(2):
                    ps = ps_s.tile([128, 512], F32, tag="s")
                    nc.tensor.matmul(ps, eat_sb[:, 128 * rb:128 * (rb + 1)],
                                     wa_sb[:, 512 * o2:512 * (o2 + 1)],
                                     start=True, stop=True)
                    nc.vector.tensor_add(h_sb[:, rb, 512 * o2:512 * (o2 + 1)],
                                         pos_sb[:, rb, 512 * o2:512 * (o2 + 1)], ps)

            # warm up the ACT exp/ln table set
            warm = sm.tile([128, 1], F32, tag="s1")
            nc.vector.memset(warm, 1.0)
            nc.scalar.activation(warm, warm, AF.Ln, bias=eps_sb, scale=1.0)
            nc.scalar.activation(warm, warm, AF.Exp, bias=0.0, scale=-0.5)

            def layernorm_to_xT(g_src, b_src, xT):
                """LN(h) with affine (g,b), transposed into xT [128, NC_, RLOC] bf16.

                All 4 row-blocks' variances go through ONE Ln and ONE Exp so the
                ACT table set switches at most twice per LN call (Ln lives in a
                different spline table set than Exp/Relu/Copy)."""
                g_sb = prm.tile([128, NC_], F32, tag="lng")
                b_sb = prm.tile([128, NC_], F32, tag="lnb")
                nc.sync.dma_start(g_sb, g_src.rearrange("(c p) -> p c", p=128))
                nc.sync.dma_start(b_sb, b_src.rearrange("(c p) -> p c", p=128))
                mvall = sm.tile([128, NRB, 2], F32, tag="mv")
                for rb in range(NRB):
                    stats = sm.tile([128, 2, 6], F32, tag="st")
                    nc.vector.bn_stats(stats[:, 0, :], h_sb[:, rb, 0:512])
                    nc.vector.bn_stats(stats[:, 1, :], h_sb[:, rb, 512:1024])
                    nc.vector.bn_aggr(mvall[:, rb, :], stats)
                rstd = sm.tile([128, NRB], F32, tag="rstd")
                nc.scalar.activation(rstd, mvall[:, :, 1], AF.Ln, bias=eps_sb, scale=1.0)
                nc.scalar.activation(rstd, rstd, AF.Exp, bias=0.0, scale=-0.5)
                for rb in range(NRB):
                    xc = xcp.tile([128, D], F32, tag="xc")
                    nc.vector.tensor_scalar(xc, h_sb[:, rb, :], mvall[:, rb, 0:1],
                                            rstd[:, rb:rb + 1],
                                            ALU.subtract, ALU.mult)
                    for c in range(NC_):
                        tp = ps_s.tile([128, 128], F32, tag="s")
                        nc.tensor.transpose(tp, xc[:, 128 * c:128 * (c + 1)], ident)
                        nc.vector.tensor_scalar(
                            xT[:, c, 128 * rb:128 * (rb + 1)], tp,
                            g_sb[:, c:c + 1], b_sb[:, c:c + 1], ALU.mult, ALU.add)

            def load_w(src2d, tag="w2mb", dtype=BF):
                w = wbig.tile([128, NC_, src2d.shape[1]], dtype, tag=tag)
                nc.sync.dma_start(w, src2d.rearrange("(c p) n -> p c n", p=128))
                return w

            def attn_pass_pair(i, kt, vt, mask_sb, av0, jbase, mask_eng):
                """One attention pass (4 rk slots) for head pair (2i, 2i+1).

                Causal suffix structure: key slot j only scores/累 the query
                columns 128j:512 (ascending interleaved blocks make this shape
                identical for the local and remote pass on every core).  The
                first 128 columns of each suffix get a data-driven mask: the
                own-diagonal triangle locally; all-ones / all-zeros remotely.
                The two heads' score matmuls contract over disjoint partition
                halves of kT/qT, so the PE runs them concurrently."""
                h0, h1 = 2 * i, 2 * i + 1
                expTP = expp.tile([128, 2, 4, RLOC], BF, tag="exp")
                for j in range(4):
                    c0 = 128 * j
                    stp = ps_s.tile([128, 2, RLOC], F32, tag="s")
                    nc.tensor.matmul(stp[:, 0, c0:], kt[0:64, i, c0:c0 + 128],
                                     qT_sb[0:64, i, c0:], start=True, stop=True)
                    nc.tensor.matmul(stp[:, 1, c0:], kt[64:128, i, c0:c0 + 128],
                                     qT_sb[64:128, i, c0:], start=True, stop=True)
                    nc.scalar.activation(expTP[:, :, j, c0:], stp[:, :, c0:], AF.Exp)
                # mask the first 128 cols of every suffix (strided diag AP:
                # element (p, h, j, 128j + t) of expTP, stride 640 over j)
                full = expTP[:, :, :, :]
                for h01 in range(2):
                    dg = bass.AP(tensor=full.tensor,
                                 offset=full.offset + h01 * 4 * RLOC,
                                 ap=[list(full.ap[0]), [640, 4], [1, 128]])
                    mask_eng.tensor_tensor(dg, dg,
                                           mask_sb[:, jbase:jbase + 4, :], ALU.mult)
                for j in range(4):
                    c0 = 128 * j
                    nc.tensor.matmul(av0[:, 0, c0:], vt[:, j, h0, :],
                                     expTP[:, 0, j, c0:], start=(j == 0), stop=(j == 3))
                    nc.tensor.matmul(av0[:, 1, c0:], vt[:, j, h1, :],
                                     expTP[:, 1, j, c0:], start=(j == 0), stop=(j == 3))

            for l in range(L):
                # per-layer exchange: ship the 0.5MB fp8 LN1 output (gpsimd
                # casting DMAs) instead of 2MB of K/V — the AllGather starts
                # BEFORE the K/V projections and hides under own-row K/V/Q
                # work; each core re-projects the partner's K/V locally.
                kv_in = dr.tile([KSEG], F8, tag="kvin", name=f"kv_in_{l}")
                kv_all = dr.tile([2 * KSEG], F8, tag="kvall", name=f"kv_all_{l}")

                # ---- LN1 -> xT, bounce + AllGather of xT ----
                xT = xtp.tile([128, NC_, RLOC], BF, tag="xt")
                layernorm_to_xT(ln1g_in[ds(D * l, D)], ln1b_in[ds(D * l, D)], xT)
                nc.gpsimd.dma_start(
                    kv_in[:].rearrange("(c p r) -> p c r", c=NC_, p=128), xT)
                nc.gpsimd.collective_compute(
                    "AllGather", ALU.bypass,
                    replica_groups=[[0, 1], [2, 3], [4, 5], [6, 7]],
                    ins=[kv_in[:]], outs=[kv_all[:]])

                def k_proj(wk_t, x_t, out_t):
                    for o in range(NC_):
                        ps = ps_s.tile([128, 512], F32, tag="s")
                        for c in range(NC_):
                            nc.tensor.matmul(ps, wk_t[:, c, 128 * o:128 * (o + 1)],
                                             x_t[:, c, :],
                                             start=(c == 0), stop=(c == NC_ - 1))
                        nc.scalar.copy(out_t[:, o, :], ps)

                def v_proj(wv_t, x_t, out_t):
                    for rb in range(NRB):
                        for o2 in range(2):
                            ps = ps_s.tile([128, 512], F32, tag="s")
                            for c in range(NC_):
                                nc.tensor.matmul(ps, x_t[:, c, 128 * rb:128 * (rb + 1)],
                                                 wv_t[:, c, 512 * o2:512 * (o2 + 1)],
                                                 start=(c == 0), stop=(c == NC_ - 1))
                            nc.scalar.copy(
                                out_t[:, rb, 8 * o2:8 * (o2 + 1), 0:HD],
                                ps.rearrange("p (hh e) -> p hh e", hh=8))

                # ---- own K/V/Q projections (overlap the AllGather) ----
                wk_sb = load_w(wk_in[D * l:D * (l + 1)])
                k_proj(wk_sb, xT, kst)
                wv_sb = load_w(wv_in[D * l:D * (l + 1)])
                v_proj(wv_sb, xT, v_loc)
                wq_sb = load_w(wq_in[D * l:D * (l + 1)])
                for o in range(NC_):
                    ps = ps_s.tile([128, 512], F32, tag="s")
                    for c in range(NC_):
                        nc.tensor.matmul(ps, wq_sb[:, c, 128 * o:128 * (o + 1)],
                                         xT[:, c, :], start=(c == 0), stop=(c == NC_ - 1))
                    nc.scalar.mul(qT_sb[:, o, :], ps, float(SCALE))

                # ---- attention pass 1: own K/V (no AllGather dependency) ----
                mask_sb = ytp.tile([128, 8, 128], BF, tag="yt")
                nc.sync.dma_start(mask_sb, masks_in[:, :, :])
                for i in range(H // 2):
                    pa = ps_av.tile([HD + 1, 2, RLOC], F32, tag="av")
                    attn_pass_pair(i, kst, v_loc, mask_sb, pa, 0, nc.vector)
                    nc.vector.tensor_copy(p1_sb[:, 2 * i:2 * i + 2, :], pa)

                # ---- fetch partner xT (fp8 -> bf16), re-project its K/V ----
                xT_rem = xtp.tile([128, NC_, RLOC], BF, tag="xt")
                nc.gpsimd.dma_start(
                    xT_rem,
                    kv_all[ds(rem_base, KSEG)].rearrange("(c p r) -> p c r",
                                                         c=NC_, p=128))
                wk_sb2 = load_w(wk_in[D * l:D * (l + 1)])
                k_proj(wk_sb2, xT_rem, kT_rem)
                wv_sb2 = load_w(wv_in[D * l:D * (l + 1)])
                v_proj(wv_sb2, xT_rem, v_rem)

                # ---- attention pass 2: partner K/V, combine, normalize ----
                def finalize_pair(i, av):
                    """Combine both passes in PSUM, then normalize the two
                    heads with one 2-vector reciprocal and two K=1 broadcast
                    matmuls whose out rows 0:64 / 64:128 carry 1/sum for head
                    2i / 2i+1.  (All-SBUF operands of any op share a base
                    partition — walrus requires it.)"""
                    nc.vector.tensor_tensor(av, av, p1_sb[:, 2 * i:2 * i + 2, :],
                                            ALU.add)
                    s2 = sm.tile([1, 2, RLOC], F32, tag="s1h", bufs=1)
                    nc.vector.tensor_copy(s2, av[HD:HD + 1, :, :])
                    nc.vector.reciprocal_approx_fast(s2, s2)
                    rc1 = sm.tile([1, 2, RLOC], BF, tag="rc1", bufs=1)
                    nc.gpsimd.tensor_copy(rc1, s2)
                    bc_sb = sm.tile([128, 2, RLOC], BF, tag="bcsb", bufs=1)
                    nc.gpsimd.partition_broadcast(bc_sb, rc1)
                    nc.vector.tensor_tensor(oT_sb[0:64, i, :], av[0:HD, 0, :],
                                            bc_sb[0:64, 0, :], ALU.mult)
                    nc.vector.tensor_tensor(oT_sb[64:128, i, :], av[0:HD, 1, :],
                                            bc_sb[64:128, 1, :], ALU.mult)

                wo_sb = load_w(wo_in[D * l:D * (l + 1)])
                for i in range(H // 2):
                    pa = ps_av.tile([HD + 1, 2, RLOC], F32, tag="av")
                    attn_pass_pair(i, kT_rem, v_rem, mask_sb, pa, 4, nc.gpsimd)
                    finalize_pair(i, pa)

                # ---- output projection + residual ----
                for rb in range(NRB):
                    for o2 in range(2):
                        ps = ps_s.tile([128, 512], F32, tag="s")
                        for c in range(NC_):
                            nc.tensor.matmul(ps, oT_sb[:, c, 128 * rb:128 * (rb + 1)],
                                             wo_sb[:, c, 512 * o2:512 * (o2 + 1)],
                                             start=(c == 0), stop=(c == NC_ - 1))
                        hsl = h_sb[:, rb, 512 * o2:512 * (o2 + 1)]
                        nc.vector.tensor_add(hsl, hsl, ps)

                # ---- LN2 -> xT2 (bf16 — the FFN is too error-sensitive for fp8) ----
                xT2 = xtp.tile([128, NC_, RLOC], BF, tag="xt")
                layernorm_to_xT(ln2g_in[ds(D * l, D)], ln2b_in[ds(D * l, D)], xT2)

                # b2 folds into h right after LN2's reads (gpsimd, off the
                # FFN2 tail that otherwise gates the next layer's LN1)
                b2_bc = prm.tile([128, D], F32, tag="b2")
                nc.sync.dma_start(b2_bc, bcast_ap(b2_in[ds(D * l, D)]))
                for rb in range(NRB):
                    nc.gpsimd.tensor_tensor(h_sb[:, rb, :], h_sb[:, rb, :],
                                            b2_bc, ALU.add)

                # ---- FFN1: yT = relu(w1^T x + b1) ----
                b1_sb = prm.tile([128, NFO], F32, tag="b1")
                nc.sync.dma_start(b1_sb, b1_in[ds(F * l, F)].rearrange("(o p) -> p o", p=128))
                yT = ytp.tile([128, NFO, RLOC], BF, tag="yt")
                for phi in range(4):
                    w1_sb = load_w(w1_in[D * l:D * (l + 1), 1024 * phi:1024 * (phi + 1)])
                    for fo in range(8):
                        fg = 8 * phi + fo
                        ps = ps_s.tile([128, 512], F32, tag="s")
                        for c in range(NC_):
                            nc.tensor.matmul(ps, w1_sb[:, c, 128 * fo:128 * (fo + 1)],
                                             xT2[:, c, :], start=(c == 0), stop=(c == NC_ - 1))
                        nc.scalar.activation(yT[:, fg, :], ps, AF.Relu,
                                             bias=b1_sb[:, fg:fg + 1], scale=1.0)

                # ---- FFN2: h += yT^T @ w2 (+ b2) ----
                for phi in range(4):
                    w2_sb = load_w(w2_in[F * l + 1024 * phi:F * l + 1024 * (phi + 1)])
                    for rb in range(NRB):
                        for o2 in range(2):
                            ps = ps_s.tile([128, 512], F32, tag="s")
                            for c in range(NC_):
                                nc.tensor.matmul(
                                    ps, yT[:, 8 * phi + c, 128 * rb:128 * (rb + 1)],
                                    w2_sb[:, c, 512 * o2:512 * (o2 + 1)],
                                    start=(c == 0), stop=(c == NC_ - 1))
                            hsl = h_sb[:, rb, 512 * o2:512 * (o2 + 1)]
                            nc.vector.tensor_add(hsl, hsl, ps)

            # ---- final LN + decoder ----
            xTf = xtp.tile([128, NC_, RLOC], BF, tag="xt")
            layernorm_to_xT(lnfg_in[:], lnfb_in[:], xTf)
            out_sb = res.tile([128, NRB, OUT], F32)
            for rb in range(NRB):
                ps = ps_s.tile([128, OUT], F32, tag="s")
                for c in range(NC_):
                    nc.tensor.matmul(ps, xTf[:, c, 128 * rb:128 * (rb + 1)],
                                     wd_sb[:, c, :], start=(c == 0), stop=(c == NC_ - 1))
                nc.vector.tensor_add(out_sb[:, rb, :], bd_bc, ps)
            nc.sync.dma_start(out_p.rearrange("(rb p) n -> p rb n", p=128), out_sb)

    nc.compile()
    return nc


_PROGRAM = None


def _get_program():
    global _PROGRAM
    if _PROGRAM is None:
        _PROGRAM = _build_program()
    return _PROGRAM


def _bf(x):
    return np.ascontiguousarray(np.asarray(x, np.float32)).astype(ml_dtypes.bfloat16)


def _f8(x):
    return np.ascontiguousarray(np.asarray(x, np.float32)).astype(ml_dtypes.float8_e4m3)


def _prep_inputs(inputs):
    """Host-side sharding: build the per-core input maps."""
    I = {k: np.asarray(v) for k, v in inputs.items()}

    wq = _bf(I["wq"].reshape(L * D, D))
    wk = _bf(I["wk"].reshape(L * D, D))
    wv = _bf(I["wv"].reshape(L * D, D))
    wo = _bf(I["wo"].reshape(L * D, D))
    w1 = _bf(I["w1"].reshape(L * D, F))
    w2 = _bf(I["w2"].reshape(L * F, D))
    b1 = np.asarray(I["b1"].reshape(L * F), np.float32)
    b2 = np.asarray(I["b2"].reshape(L * D), np.float32)
    ln1g = np.asarray(I["ln1_g"].reshape(L * D), np.float32)
    ln1b = np.asarray(I["ln1_b"].reshape(L * D), np.float32)
    ln2g = np.asarray(I["ln2_g"].reshape(L * D), np.float32)
    ln2b = np.asarray(I["ln2_b"].reshape(L * D), np.float32)
    lnfg = np.asarray(I["lnf_g"], np.float32)
    lnfb = np.asarray(I["lnf_b"], np.float32)
    wd = _bf(I["wd"])
    bd = np.asarray(I["bd"], np.float32)

    # augmented embedding table [64, D]
    wa = np.zeros((64, D), np.float32)
    wa[0:V1] = I["emb_cat1"]
    wa[V1:V1 + V2] = I["emb_cat2"]
    wa[48] = I["w_num1"][0]
    wa[49] = I["w_num2"][0]
    wa[50] = I["bos"][0, 0]
    wa = _bf(wa)

    pos_emb = np.asarray(I["pos_emb"], np.float32)
    cat1 = np.asarray(I["tgt_cat1"])
    cat2 = np.asarray(I["tgt_cat2"])
    num1 = np.asarray(I["tgt_num1"], np.float32)
    num2 = np.asarray(I["tgt_num2"], np.float32)

    in_maps = []
    shared = dict(wq=wq, wk=wk, wv=wv, wo=wo, w1=w1, w2=w2, b1=b1, b2=b2,
                  ln1g=ln1g, ln1b=ln1b, ln2g=ln2g, ln2b=ln2b,
                  lnfg=lnfg, lnfb=lnfb, wd=wd, bd=bd, wa=wa)
    for c in range(NCORES):
        b, parity = c // 2, c % 2
        grows = np.concatenate([np.arange(128 * g, 128 * (g + 1))
                                for g in BLOCKS[parity]])        # [512] global rows
        grows_rem = np.concatenate([np.arange(128 * g, 128 * (g + 1))
                                    for g in BLOCKS[1 - parity]])
        # embedding selector EaT [64, 512]
        eat = np.zeros((64, RLOC), np.float32)
        for r, g in enumerate(grows):
            if g == 0:
                eat[50, r] = 1.0
            else:
                t = g - 1
                eat[cat1[b, t], r] = 1.0
                eat[V1 + cat2[b, t], r] = 1.0
                eat[48, r] = num1[b, t, 0]
                eat[49, r] = num2[b, t, 0]
        # shifted positional embedding [512, D]
        pos = np.zeros((RLOC, D), np.float32)
        nz = grows > 0
        pos[nz] = pos_emb[grows[nz] - 1]
        # binary causal masks for the first 128 query-cols of each key slot's
        # suffix: [128, 8, 128] — slots 0..3 local (own diagonal triangle),
        # slots 4..7 remote (all-ones when partner block < own, else zeros)
        mask = np.zeros((128, 8, 128), np.float32)
        for j in range(4):
            gk = grows[128 * j:128 * (j + 1)]        # own k-block j rows
            gq = grows[128 * j:128 * (j + 1)]        # first suffix q-block = j
            mask[:, j, :] = (gk[:, None] <= gq[None, :])
            rk = grows_rem[128 * j:128 * (j + 1)]    # partner k-block j rows
            mask[:, 4 + j, :] = (rk[:, None] <= gq[None, :])
        in_maps.append(dict(shared,
                            eat=_bf(eat), pos=pos, masks=_bf(mask)))
    return in_maps


def _unshard_output(results):
    out = np.zeros((B, S, OUT), np.float32)
    for c in range(NCORES):
        b, parity = c // 2, c % 2
        grows = np.concatenate([np.arange(128 * g, 128 * (g + 1))
                                for g in BLOCKS[parity]])
        out[b, grows] = results[c]["out"]
    return out


def kernel(**inputs):
    nc = _get_program()
    in_maps = _prep_inputs(inputs)
    res = run_bass_kernel_spmd(nc, in_maps, core_ids=list(range(NCORES)))
    return _unshard_output(res.results)


def run_traced(inputs):
    """Like kernel() but with NTFF tracing; returns (output, BassKernelResults)."""
    nc = _get_program()
    in_maps = _prep_inputs(inputs)
    res = run_bass_kernel_spmd(nc, in_maps, core_ids=list(range(NCORES)),
                               trace=True, trace_cores=list(range(NCORES)))
    return _unshard_output(res.results), res



# revision 72
# speedup vs baseline: 1.0147x; 1.0147x over previous
"""Trainium2 Bass kernel for nn_AutoReg (4-layer dense transformer, teacher forcing).

Sharding across 8 NeuronCores: data-parallel over batch (B=4 -> 4 core pairs),
sequence-split within each pair with INTERLEAVED row blocks (even core owns
S-blocks {0,2,4,6}, odd owns {1,3,5,7}).  Ascending interleaved blocks give
the local and remote attention passes the identical causal suffix structure
[512,384,256,128] on every core, so one SPMD program skips ~38% of
score/exp/AV work (even cores waste one fully-masked 128-col sub-block per
remote key slot; the masks are per-core data).

Per layer the cores exchange only the 0.5MB fp8e4m3-cast LN1 output (gpsimd
casting DMAs + pair AllGather) — the collective starts before the K/V
projections and hides entirely under own-row K/V/Q + the local attention
pass; each core then re-projects the partner's K/V locally.

Math: bf16 matmuls with fp32 PSUM accumulation; LayerNorm, softmax and the
residual stream fp32 (fp8 touches only the xT wire).  Scores are computed
transposed (sT[rk, rq]) so the softmax denominator comes out of the AV
matmul via an appended ones column; the per-rstd Ln/Exp pair is batched per
LayerNorm to avoid ACT spline-table thrashing; softmax normalization uses a
fast approximate reciprocal + a gpsimd partition_broadcast.  Engine split:
PE matmuls, ACT exp/relu/copies, DVE LN/combine/normalize, GPSIMD masks
(remote pass), reciprocal broadcast, b2 fold and all casting DMAs — nothing
that must run during the collective sits behind it on the Pool queue.
"""

import numpy as np
import ml_dtypes

import concourse.bass as bass
import concourse.bacc as bacc
import concourse.mybir as mybir
import concourse.tile as tile
from concourse.bass import ds
from concourse.bass_utils import run_bass_kernel_spmd
from concourse.masks import make_identity

# Model dims (hardcoded per the problem spec)
L, B, S, D, H, F = 4, 4, 1024, 1024, 16, 4096
V1, V2, OUT = 32, 16, 50
HD = D // H            # 64
NCORES = 8
RLOC = 512             # local rows per core
NRB = RLOC // 128      # 4 local row blocks
NC_ = D // 128         # 8 D-chunks
NFO = F // 128         # 32 F-chunks
SCALE = 1.0 / np.sqrt(HD)

# global row-block assignment per parity (interleaved): parity 0 (even
# cores) own blocks [0,2,4,6]; parity 1 own [1,3,5,7].  With ascending
# interleaved blocks, BOTH the local and remote attention passes have the
# same causal suffix structure [512,384,256,128] on every core (even cores
# compute one fully-masked 128-col sub-block per remote key slot; odd cores
# use all of it), so the SPMD program can skip ~38% of score/exp/AV work.
BLOCKS = {0: [0, 2, 4, 6], 1: [1, 3, 5, 7]}

BF = mybir.dt.bfloat16
F32 = mybir.dt.float32
F8 = mybir.dt.float8e4

KSEG = D * RLOC              # elems: kT region of one core's kv block
VSEG = RLOC * D              # elems: v region
SEG = KSEG + VSEG            # elems per rank in the AllGather


def _build_program():
    nc = bacc.Bacc("TRN2", target_bir_lowering=False)

    # ---- DRAM parameters (per-core inputs) ----
    eat_in = nc.declare_dram_parameter("eat", [64, RLOC], BF, isOutput=False)
    wa_in = nc.declare_dram_parameter("wa", [64, D], BF, isOutput=False)
    pos_in = nc.declare_dram_parameter("pos", [RLOC, D], F32, isOutput=False)
    masks_in = nc.declare_dram_parameter("masks", [128, 8, 128], BF, isOutput=False)
    wq_in = nc.declare_dram_parameter("wq", [L * D, D], BF, isOutput=False)
    wk_in = nc.declare_dram_parameter("wk", [L * D, D], BF, isOutput=False)
    wv_in = nc.declare_dram_parameter("wv", [L * D, D], BF, isOutput=False)
    wo_in = nc.declare_dram_parameter("wo", [L * D, D], BF, isOutput=False)
    w1_in = nc.declare_dram_parameter("w1", [L * D, F], BF, isOutput=False)
    w2_in = nc.declare_dram_parameter("w2", [L * F, D], BF, isOutput=False)
    b1_in = nc.declare_dram_parameter("b1", [L * F], F32, isOutput=False)
    b2_in = nc.declare_dram_parameter("b2", [L * D], F32, isOutput=False)
    ln1g_in = nc.declare_dram_parameter("ln1g", [L * D], F32, isOutput=False)
    ln1b_in = nc.declare_dram_parameter("ln1b", [L * D], F32, isOutput=False)
    ln2g_in = nc.declare_dram_parameter("ln2g", [L * D], F32, isOutput=False)
    ln2b_in = nc.declare_dram_parameter("ln2b", [L * D], F32, isOutput=False)
    lnfg_in = nc.declare_dram_parameter("lnfg", [D], F32, isOutput=False)
    lnfb_in = nc.declare_dram_parameter("lnfb", [D], F32, isOutput=False)
    wd_in = nc.declare_dram_parameter("wd", [D, OUT], BF, isOutput=False)
    bd_in = nc.declare_dram_parameter("bd", [OUT], F32, isOutput=False)
    out_p = nc.declare_dram_parameter("out", [RLOC, OUT], F32, isOutput=True)

    def bcast_ap(src_ap, p=128):
        """Partition-broadcast view of a 1-D DRAM AP."""
        return bass.AP(tensor=src_ap.tensor, offset=src_ap.offset,
                       ap=[[0, p]] + [list(x) for x in src_ap.ap])

    AF = mybir.ActivationFunctionType
    ALU = mybir.AluOpType

    with tile.TileContext(nc) as tc:
        with tc.tile_pool(name="res", bufs=1) as res, \
             tc.tile_pool(name="wbig", bufs=2) as wbig, \
             tc.tile_pool(name="yt", bufs=1) as ytp, \
             tc.tile_pool(name="xt", bufs=2) as xtp, \
             tc.tile_pool(name="expp", bufs=2) as expp, \
             tc.tile_pool(name="xc", bufs=2) as xcp, \
             tc.tile_pool(name="prm", bufs=2) as prm, \
             tc.tile_pool(name="sm", bufs=4) as sm, \
             tc.tile_pool(name="dr", bufs=1, space="DRAM") as dr, \
             tc.tile_pool(name="ps_s", bufs=2, space="PSUM") as ps_s, \
             tc.tile_pool(name="ps_av", bufs=2, space="PSUM") as ps_av:

            # ---- resident tiles ----
            h_sb = res.tile([128, NRB, D], F32)            # residual stream
            kst = res.tile([128, NC_, RLOC], BF)           # own K^T
            kT_rem = res.tile([128, NC_, RLOC], BF)        # partner K^T
            v_loc = res.tile([128, NRB, H, HD + 1], BF)    # own V + ones col
            v_rem = res.tile([128, NRB, H, HD + 1], BF)    # partner V + ones col
            qT_sb = res.tile([128, NC_, RLOC], BF)
            oT_sb = res.tile([128, NC_, RLOC], BF)
            p1_sb = res.tile([HD + 1, H, RLOC], BF)        # pass-1 partial (o|sum)
            eat_sb = res.tile([64, RLOC], BF)
            wa_sb = res.tile([64, D], BF)
            ident = res.tile([128, 128], BF)
            ones1 = res.tile([1, 64], BF)
            wd_sb = res.tile([128, NC_, OUT], BF)
            bd_bc = res.tile([128, OUT], F32)
            eps_sb = res.tile([128, 1], F32)

            make_identity(nc, ident)
            nc.vector.memset(eps_sb, 1e-6)
            nc.vector.memset(ones1, 1.0)
            nc.vector.memset(v_loc[:, :, :, HD:HD + 1], 1.0)
            nc.vector.memset(v_rem[:, :, :, HD:HD + 1], 1.0)
            nc.sync.dma_start(eat_sb, eat_in[:, :])
            nc.sync.dma_start(wa_sb, wa_in[:, :])
            nc.sync.dma_start(wd_sb, wd_in.rearrange("(c p) n -> p c n", p=128))
            nc.sync.dma_start(bd_bc, bcast_ap(bd_in[:]))

            # dynamic base: partner's segment offset in the pair AllGather
            # output (register on the Pool engine — the fetches are gpsimd
            # casting DMAs)
            pid = nc.gpsimd.partition_id()
            par = pid - (pid // 2) * 2
            rem_base = (1 - par) * KSEG

            # ---- embedding: h = EaT^T @ Wa + pos ----
            pos_sb = wbig.tile([128, NRB, D], F32, tag="w2mb")
            pos_v = pos_in.rearrange("(rb p) d -> rb p d", p=128)
            for rb in range(NRB):
                nc.sync.dma_start(pos_sb[:, rb, :], pos_v[rb])
            for rb in range(NRB):
                for o2 in range(2):
                    ps = ps_s.tile([128, 512], F32, tag="s")
                    nc.tensor.matmul(ps, eat_sb[:, 128 * rb:128 * (rb + 1)],
                                     wa_sb[:, 512 * o2:512 * (o2 + 1)],
                                     start=True, stop=True)
                    nc.vector.tensor_add(h_sb[:, rb, 512 * o2:512 * (o2 + 1)],
                                         pos_sb[:, rb, 512 * o2:512 * (o2 + 1)], ps)

            # warm up the ACT exp/ln table set
            warm = sm.tile([128, 1], F32, tag="s1")
            nc.vector.memset(warm, 1.0)
            nc.scalar.activation(warm, warm, AF.Ln, bias=eps_sb, scale=1.0)
            nc.scalar.activation(warm, warm, AF.Exp, bias=0.0, scale=-0.5)

            def layernorm_to_xT(g_src, b_src, xT):
                """LN(h) with affine (g,b), transposed into xT [128, NC_, RLOC] bf16.

                All 4 row-blocks' variances go through ONE Ln and ONE Exp so the
                ACT table set switches at most twice per LN call (Ln lives in a
                different spline table set than Exp/Relu/Copy)."""
                g_sb = prm.tile([128, NC_], F32, tag="lng")
                b_sb = prm.tile([128, NC_], F32, tag="lnb")
                nc.sync.dma_start(g_sb, g_src.rearrange("(c p) -> p c", p=128))
                nc.sync.dma_start(b_sb, b_src.rearrange("(c p) -> p c", p=128))
                mvall = sm.tile([128, NRB, 2], F32, tag="mv")
                for rb in range(NRB):
                    stats = sm.tile([128, 2, 6], F32, tag="st")
                    nc.vector.bn_stats(stats[:, 0, :], h_sb[:, rb, 0:512])
                    nc.vector.bn_stats(stats[:, 1, :], h_sb[:, rb, 512:1024])
                    nc.vector.bn_aggr(mvall[:, rb, :], stats)
                rstd = sm.tile([128, NRB], F32, tag="rstd")
                nc.scalar.activation(rstd, mvall[:, :, 1], AF.Ln, bias=eps_sb, scale=1.0)
                nc.scalar.activation(rstd, rstd, AF.Exp, bias=0.0, scale=-0.5)
                for rb in range(NRB):
                    xc = xcp.tile([128, D], BF, tag="xc")
                    nc.vector.tensor_scalar(xc, h_sb[:, rb, :], mvall[:, rb, 0:1],
                                            rstd[:, rb:rb + 1],
                                            ALU.subtract, ALU.mult)
                    for c in range(NC_):
                        tp = ps_s.tile([128, 128], BF, tag="s")
                        nc.tensor.transpose(tp, xc[:, 128 * c:128 * (c + 1)], ident)
                        nc.vector.tensor_scalar(
                            xT[:, c, 128 * rb:128 * (rb + 1)], tp,
                            g_sb[:, c:c + 1], b_sb[:, c:c + 1], ALU.mult, ALU.add)

            def load_w(src2d, tag="w2mb", dtype=BF):
                w = wbig.tile([128, NC_, src2d.shape[1]], dtype, tag=tag)
                nc.sync.dma_start(w, src2d.rearrange("(c p) n -> p c n", p=128))
                return w

            def attn_pass_pair(i, kt, vt, mask_sb, av0, jbase, mask_eng):
                """One attention pass (4 rk slots) for head pair (2i, 2i+1).

                Causal suffix structure: key slot j only scores/累 the query
                columns 128j:512 (ascending interleaved blocks make this shape
                identical for the local and remote pass on every core).  The
                first 128 columns of each suffix get a data-driven mask: the
                own-diagonal triangle locally; all-ones / all-zeros remotely.
                The two heads' score matmuls contract over disjoint partition
                halves of kT/qT, so the PE runs them concurrently."""
                h0, h1 = 2 * i, 2 * i + 1
                expTP = expp.tile([128, 2, 4, RLOC], BF, tag="exp")
                for j in range(4):
                    c0 = 128 * j
                    stp = ps_s.tile([128, 2, RLOC], F32, tag="s")
                    nc.tensor.matmul(stp[:, 0, c0:], kt[0:64, i, c0:c0 + 128],
                                     qT_sb[0:64, i, c0:], start=True, stop=True)
                    nc.tensor.matmul(stp[:, 1, c0:], kt[64:128, i, c0:c0 + 128],
                                     qT_sb[64:128, i, c0:], start=True, stop=True)
                    nc.scalar.activation(expTP[:, :, j, c0:], stp[:, :, c0:], AF.Exp)
                # mask the first 128 cols of every suffix (strided diag AP:
                # element (p, h, j, 128j + t) of expTP, stride 640 over j)
                full = expTP[:, :, :, :]
                for h01 in range(2):
                    dg = bass.AP(tensor=full.tensor,
                                 offset=full.offset + h01 * 4 * RLOC,
                                 ap=[list(full.ap[0]), [640, 4], [1, 128]])
                    mask_eng.tensor_tensor(dg, dg,
                                           mask_sb[:, jbase:jbase + 4, :], ALU.mult)
                for j in range(4):
                    c0 = 128 * j
                    nc.tensor.matmul(av0[:, 0, c0:], vt[:, j, h0, :],
                                     expTP[:, 0, j, c0:], start=(j == 0), stop=(j == 3))
                    nc.tensor.matmul(av0[:, 1, c0:], vt[:, j, h1, :],
                                     expTP[:, 1, j, c0:], start=(j == 0), stop=(j == 3))

            for l in range(L):
                # per-layer exchange: ship the 0.5MB fp8 LN1 output (gpsimd
                # casting DMAs) instead of 2MB of K/V — the AllGather starts
                # BEFORE the K/V projections and hides under own-row K/V/Q
                # work; each core re-projects the partner's K/V locally.
                kv_in = dr.tile([KSEG], F8, tag="kvin", name=f"kv_in_{l}")
                kv_all = dr.tile([2 * KSEG], F8, tag="kvall", name=f"kv_all_{l}")

                # ---- LN1 -> xT, bounce + AllGather of xT ----
                xT = xtp.tile([128, NC_, RLOC], BF, tag="xt")
                layernorm_to_xT(ln1g_in[ds(D * l, D)], ln1b_in[ds(D * l, D)], xT)
                nc.gpsimd.dma_start(
                    kv_in[:].rearrange("(c p r) -> p c r", c=NC_, p=128), xT)
                nc.gpsimd.collective_compute(
                    "AllGather", ALU.bypass,
                    replica_groups=[[0, 1], [2, 3], [4, 5], [6, 7]],
                    ins=[kv_in[:]], outs=[kv_all[:]])

                def k_proj(wk_t, x_t, out_t):
                    for o in range(NC_):
                        ps = ps_s.tile([128, 512], F32, tag="s")
                        for c in range(NC_):
                            nc.tensor.matmul(ps, wk_t[:, c, 128 * o:128 * (o + 1)],
                                             x_t[:, c, :],
                                             start=(c == 0), stop=(c == NC_ - 1))
                        nc.scalar.copy(out_t[:, o, :], ps)

                def v_proj(wv_t, x_t, out_t):
                    for rb in range(NRB):
                        for o2 in range(2):
                            ps = ps_s.tile([128, 512], F32, tag="s")
                            for c in range(NC_):
                                nc.tensor.matmul(ps, x_t[:, c, 128 * rb:128 * (rb + 1)],
                                                 wv_t[:, c, 512 * o2:512 * (o2 + 1)],
                                                 start=(c == 0), stop=(c == NC_ - 1))
                            nc.scalar.copy(
                                out_t[:, rb, 8 * o2:8 * (o2 + 1), 0:HD],
                                ps.rearrange("p (hh e) -> p hh e", hh=8))

                # ---- own K/V/Q projections (overlap the AllGather) ----
                wk_sb = load_w(wk_in[D * l:D * (l + 1)])
                k_proj(wk_sb, xT, kst)
                wv_sb = load_w(wv_in[D * l:D * (l + 1)])
                v_proj(wv_sb, xT, v_loc)
                wq_sb = load_w(wq_in[D * l:D * (l + 1)])
                for o in range(NC_):
                    ps = ps_s.tile([128, 512], F32, tag="s")
                    for c in range(NC_):
                        nc.tensor.matmul(ps, wq_sb[:, c, 128 * o:128 * (o + 1)],
                                         xT[:, c, :], start=(c == 0), stop=(c == NC_ - 1))
                    nc.scalar.mul(qT_sb[:, o, :], ps, float(SCALE))

                # ---- attention pass 1: own K/V (no AllGather dependency) ----
                mask_sb = ytp.tile([128, 8, 128], BF, tag="yt")
                nc.sync.dma_start(mask_sb, masks_in[:, :, :])
                for i in range(H // 2):
                    pa = ps_av.tile([HD + 1, 2, RLOC], F32, tag="av")
                    attn_pass_pair(i, kst, v_loc, mask_sb, pa, 0, nc.vector)
                    nc.vector.tensor_copy(p1_sb[:, 2 * i:2 * i + 2, :], pa)

                # ---- fetch partner xT (fp8 -> bf16), re-project its K/V ----
                xT_rem = xtp.tile([128, NC_, RLOC], BF, tag="xt")
                nc.gpsimd.dma_start(
                    xT_rem,
                    kv_all[ds(rem_base, KSEG)].rearrange("(c p r) -> p c r",
                                                         c=NC_, p=128))
                wk_sb2 = load_w(wk_in[D * l:D * (l + 1)])
                k_proj(wk_sb2, xT_rem, kT_rem)
                wv_sb2 = load_w(wv_in[D * l:D * (l + 1)])
                v_proj(wv_sb2, xT_rem, v_rem)

                # ---- attention pass 2: partner K/V, combine, normalize ----
                def finalize_pair(i, av):
                    """Combine both passes in PSUM, then normalize the two
                    heads with one 2-vector reciprocal and two K=1 broadcast
                    matmuls whose out rows 0:64 / 64:128 carry 1/sum for head
                    2i / 2i+1.  (All-SBUF operands of any op share a base
                    partition — walrus requires it.)"""
                    nc.vector.tensor_tensor(av, av, p1_sb[:, 2 * i:2 * i + 2, :],
                                            ALU.add)
                    s2 = sm.tile([1, 2, RLOC], F32, tag="s1h", bufs=1)
                    nc.vector.tensor_copy(s2, av[HD:HD + 1, :, :])
                    nc.vector.reciprocal_approx_fast(s2, s2)
                    rc1 = sm.tile([1, 2, RLOC], BF, tag="rc1", bufs=1)
                    nc.gpsimd.tensor_copy(rc1, s2)
                    bc_sb = sm.tile([128, 2, RLOC], BF, tag="bcsb", bufs=1)
                    nc.gpsimd.partition_broadcast(bc_sb, rc1)
                    nc.vector.tensor_tensor(oT_sb[0:64, i, :], av[0:HD, 0, :],
                                            bc_sb[0:64, 0, :], ALU.mult)
                    nc.vector.tensor_tensor(oT_sb[64:128, i, :], av[0:HD, 1, :],
                                            bc_sb[64:128, 1, :], ALU.mult)

                wo_sb = load_w(wo_in[D * l:D * (l + 1)])
                for i in range(H // 2):
                    pa = ps_av.tile([HD + 1, 2, RLOC], F32, tag="av")
                    attn_pass_pair(i, kT_rem, v_rem, mask_sb, pa, 4, nc.gpsimd)
                    finalize_pair(i, pa)

                # ---- output projection + residual ----
                for rb in range(NRB):
                    for o2 in range(2):
                        ps = ps_s.tile([128, 512], F32, tag="s")
                        for c in range(NC_):
                            nc.tensor.matmul(ps, oT_sb[:, c, 128 * rb:128 * (rb + 1)],
                                             wo_sb[:, c, 512 * o2:512 * (o2 + 1)],
                                             start=(c == 0), stop=(c == NC_ - 1))
                        hsl = h_sb[:, rb, 512 * o2:512 * (o2 + 1)]
                        nc.vector.tensor_add(hsl, hsl, ps)

                # ---- LN2 -> xT2 (bf16 — the FFN is too error-sensitive for fp8) ----
                xT2 = xtp.tile([128, NC_, RLOC], BF, tag="xt")
                layernorm_to_xT(ln2g_in[ds(D * l, D)], ln2b_in[ds(D * l, D)], xT2)

                # b2 folds into h right after LN2's reads (gpsimd, off the
                # FFN2 tail that otherwise gates the next layer's LN1)
                b2_bc = prm.tile([128, D], F32, tag="b2")
                nc.sync.dma_start(b2_bc, bcast_ap(b2_in[ds(D * l, D)]))
                for rb in range(NRB):
                    nc.gpsimd.tensor_tensor(h_sb[:, rb, :], h_sb[:, rb, :],
                                            b2_bc, ALU.add)

                # ---- FFN1: yT = relu(w1^T x + b1) ----
                b1_sb = prm.tile([128, NFO], F32, tag="b1")
                nc.sync.dma_start(b1_sb, b1_in[ds(F * l, F)].rearrange("(o p) -> p o", p=128))
                yT = ytp.tile([128, NFO, RLOC], BF, tag="yt")
                for phi in range(4):
                    w1_sb = load_w(w1_in[D * l:D * (l + 1), 1024 * phi:1024 * (phi + 1)])
                    for fo in range(8):
                        fg = 8 * phi + fo
                        ps = ps_s.tile([128, 512], F32, tag="s")
                        for c in range(NC_):
                            nc.tensor.matmul(ps, w1_sb[:, c, 128 * fo:128 * (fo + 1)],
                                             xT2[:, c, :], start=(c == 0), stop=(c == NC_ - 1))
                        nc.scalar.activation(yT[:, fg, :], ps, AF.Relu,
                                             bias=b1_sb[:, fg:fg + 1], scale=1.0)

                # ---- FFN2: h += yT^T @ w2 (+ b2) ----
                for phi in range(4):
                    w2_sb = load_w(w2_in[F * l + 1024 * phi:F * l + 1024 * (phi + 1)])
                    for rb in range(NRB):
                        for o2 in range(2):
                            ps = ps_s.tile([128, 512], F32, tag="s")
                            for c in range(NC_):
                                nc.tensor.matmul(
                                    ps, yT[:, 8 * phi + c, 128 * rb:128 * (rb + 1)],
                                    w2_sb[:, c, 512 * o2:512 * (o2 + 1)],
                                    start=(c == 0), stop=(c == NC_ - 1))
                            hsl = h_sb[:, rb, 512 * o2:512 * (o2 + 1)]
                            nc.vector.tensor_add(hsl, hsl, ps)

            # ---- final LN + decoder ----
            xTf = xtp.tile([128, NC_, RLOC], BF, tag="xt")
            layernorm_to_xT(lnfg_in[:], lnfb_in[:], xTf)
            out_sb = res.tile([128, NRB, OUT], F32)
            for rb in range(NRB):
                ps = ps_s.tile([128, OUT], F32, tag="s")
                for c in range(NC_):
                    nc.tensor.matmul(ps, xTf[:, c, 128 * rb:128 * (rb + 1)],
                                     wd_sb[:, c, :], start=(c == 0), stop=(c == NC_ - 1))
                nc.vector.tensor_add(out_sb[:, rb, :], bd_bc, ps)
            nc.sync.dma_start(out_p.rearrange("(rb p) n -> p rb n", p=128), out_sb)

    nc.compile()
    return nc


_PROGRAM = None


def _get_program():
    global _PROGRAM
    if _PROGRAM is None:
        _PROGRAM = _build_program()
    return _PROGRAM


def _bf(x):
    return np.ascontiguousarray(np.asarray(x, np.float32)).astype(ml_dtypes.bfloat16)


def _f8(x):
    return np.ascontiguousarray(np.asarray(x, np.float32)).astype(ml_dtypes.float8_e4m3)


def _prep_inputs(inputs):
    """Host-side sharding: build the per-core input maps."""
    I = {k: np.asarray(v) for k, v in inputs.items()}

    wq = _bf(I["wq"].reshape(L * D, D))
    wk = _bf(I["wk"].reshape(L * D, D))
    wv = _bf(I["wv"].reshape(L * D, D))
    wo = _bf(I["wo"].reshape(L * D, D))
    w1 = _bf(I["w1"].reshape(L * D, F))
    w2 = _bf(I["w2"].reshape(L * F, D))
    b1 = np.asarray(I["b1"].reshape(L * F), np.float32)
    b2 = np.asarray(I["b2"].reshape(L * D), np.float32)
    ln1g = np.asarray(I["ln1_g"].reshape(L * D), np.float32)
    ln1b = np.asarray(I["ln1_b"].reshape(L * D), np.float32)
    ln2g = np.asarray(I["ln2_g"].reshape(L * D), np.float32)
    ln2b = np.asarray(I["ln2_b"].reshape(L * D), np.float32)
    lnfg = np.asarray(I["lnf_g"], np.float32)
    lnfb = np.asarray(I["lnf_b"], np.float32)
    wd = _bf(I["wd"])
    bd = np.asarray(I["bd"], np.float32)

    # augmented embedding table [64, D]
    wa = np.zeros((64, D), np.float32)
    wa[0:V1] = I["emb_cat1"]
    wa[V1:V1 + V2] = I["emb_cat2"]
    wa[48] = I["w_num1"][0]
    wa[49] = I["w_num2"][0]
    wa[50] = I["bos"][0, 0]
    wa = _bf(wa)

    pos_emb = np.asarray(I["pos_emb"], np.float32)
    cat1 = np.asarray(I["tgt_cat1"])
    cat2 = np.asarray(I["tgt_cat2"])
    num1 = np.asarray(I["tgt_num1"], np.float32)
    num2 = np.asarray(I["tgt_num2"], np.float32)

    in_maps = []
    shared = dict(wq=wq, wk=wk, wv=wv, wo=wo, w1=w1, w2=w2, b1=b1, b2=b2,
                  ln1g=ln1g, ln1b=ln1b, ln2g=ln2g, ln2b=ln2b,
                  lnfg=lnfg, lnfb=lnfb, wd=wd, bd=bd, wa=wa)
    for c in range(NCORES):
        b, parity = c // 2, c % 2
        grows = np.concatenate([np.arange(128 * g, 128 * (g + 1))
                                for g in BLOCKS[parity]])        # [512] global rows
        grows_rem = np.concatenate([np.arange(128 * g, 128 * (g + 1))
                                    for g in BLOCKS[1 - parity]])
        # embedding selector EaT [64, 512]
        eat = np.zeros((64, RLOC), np.float32)
        for r, g in enumerate(grows):
            if g == 0:
                eat[50, r] = 1.0
            else:
                t = g - 1
                eat[cat1[b, t], r] = 1.0
                eat[V1 + cat2[b, t], r] = 1.0
                eat[48, r] = num1[b, t, 0]
                eat[49, r] = num2[b, t, 0]
        # shifted positional embedding [512, D]
        pos = np.zeros((RLOC, D), np.float32)
        nz = grows > 0
        pos[nz] = pos_emb[grows[nz] - 1]
        # binary causal masks for the first 128 query-cols of each key slot's
        # suffix: [128, 8, 128] — slots 0..3 local (own diagonal triangle),
        # slots 4..7 remote (all-ones when partner block < own, else zeros)
        mask = np.zeros((128, 8, 128), np.float32)
        for j in range(4):
            gk = grows[128 * j:128 * (j + 1)]        # own k-block j rows
            gq = grows[128 * j:128 * (j + 1)]        # first suffix q-block = j
            mask[:, j, :] = (gk[:, None] <= gq[None, :])
            rk = grows_rem[128 * j:128 * (j + 1)]    # partner k-block j rows
            mask[:, 4 + j, :] = (rk[:, None] <= gq[None, :])
        in_maps.append(dict(shared,
                            eat=_bf(eat), pos=pos, masks=_bf(mask)))
    return in_maps


def _unshard_output(results):
    out = np.zeros((B, S, OUT), np.float32)
    for c in range(NCORES):
        b, parity = c // 2, c % 2
        grows = np.concatenate([np.arange(128 * g, 128 * (g + 1))
                                for g in BLOCKS[parity]])
        out[b, grows] = results[c]["out"]
    return out


def kernel(**inputs):
    nc = _get_program()
    in_maps = _prep_inputs(inputs)
    res = run_bass_kernel_spmd(nc, in_maps, core_ids=list(range(NCORES)))
    return _unshard_output(res.results)


def run_traced(inputs):
    """Like kernel() but with NTFF tracing; returns (output, BassKernelResults)."""
    nc = _get_program()
    in_maps = _prep_inputs(inputs)
    res = run_bass_kernel_spmd(nc, in_maps, core_ids=list(range(NCORES)),
                               trace=True, trace_cores=list(range(NCORES)))
    return _unshard_output(res.results), res

